# revision 4
# baseline (speedup 1.0000x reference)
"""NeRF volumetric alpha-compositing kernel for Trainium2 (Bass/Tile).

Full inputs:  rgbo [131072, 128, 4] f32, depth [131072, 128] f32.
Full output:  [131072, 3] f32.

Sharding: data-parallel over rays, 8 cores x 16384 rays.

Per-core algorithm, ray-per-partition layout (BLOCK=128 rays on partitions,
T rays per partition per superblock, S=128 samples each on the free dim):

  delta[s] = depth[s+1]-depth[s]; m[s] = opacity[s]*delta[s]; m[S-1] = 0
  cs       = inclusive_cumsum(m) over the whole T*S free extent (one scan;
             zeroing the last-sample sentinel keeps rays within a partition
             row from poisoning each other)
  te[j]    = exp(-cs[j-1]), te[0] = 1        (ACT Exp, scale=-1)
  b'[t]    = 0.5*exp(+cs[t*S-1])             (per-ray un-normalizer; exp
             bias=ln(1/2) folds the sigmoid->tanh affine)
  w~[j]    = te[j]-te[j+1]; w~[t,S-1] = te[t*S+S-1]   (bf16)
  g[c]     = tanh(0.5*rgb_c)                 (bf16; sigmoid(x) =
             0.5+0.5*tanh(x/2) keeps ACT on one table set with Exp)
  S_c[t]   = sum_s w~[t,s]*g[c][t,s]         (bf16 muls + segmented reduce)
  out[t,c] = S_c[t]*b'[t] + 0.5

The per-ray transmittance factor exp(+cs[ray start-1]) cancels the cross-ray
accumulation of the single long scan; sum_s w~ telescopes to 1/b exactly, so
the sigmoid "+0.5" term reduces to the constant 0.5.

Engine split: scan/w~/muls/reduce on DVE, delta/m/finals on GPSIMD,
Exp+Tanh on ACT (one table set - no ACT_TABLE_LOAD churn), input DMA on
sync, output DMA on scalar (second HWDGE ring).
"""

from contextlib import ExitStack
from math import log as _ln

import numpy as np

import concourse.bass as bass
import concourse.tile as tile
from concourse import bacc, mybir
from concourse.bass_utils import run_bass_kernel_spmd

N_RAYS = 131072
S = 128
N_CORES = 8
NC_RAYS = N_RAYS // N_CORES  # 16384 rays per core
BLOCK = 128                  # rays per partition-block
F32 = mybir.dt.float32
BF16 = mybir.dt.bfloat16
LN_HALF = _ln(0.5)


def build_nerf_bass(
    n_rays: int = NC_RAYS,
    t_blocks: int = 8,
    bufs: int = 2,
    gpsimd_dm: bool = True,
    gpsimd_finals: bool = True,
    tanh4: bool = True,
    fold: bool = False,
    reduce_stt: bool = False,
    out_dma: str = "scalar",
) -> bass.Bass:
    T = t_blocks
    SUPER = BLOCK * T
    assert n_rays % SUPER == 0
    n_super = n_rays // SUPER
    N = S * T  # free extent per partition

    nc = bacc.Bacc("TRN2", target_bir_lowering=False, debug=False)
    rgbo_h = nc.declare_dram_parameter("rgbo", [n_rays, S, 4], F32, isOutput=False)
    depth_h = nc.declare_dram_parameter("depth", [n_rays, S], F32, isOutput=False)
    out_h = nc.declare_dram_parameter("out", [n_rays, 3], F32, isOutput=True)

    rgbo_ap = rgbo_h.ap()
    depth_ap = depth_h.ap()
    out_ap = out_h.ap()

    with ExitStack() as ctx:
        tc = ctx.enter_context(tile.TileContext(nc))
        p_in = ctx.enter_context(tc.tile_pool(name="inp", bufs=bufs))
        p_mid = ctx.enter_context(tc.tile_pool(name="mid", bufs=bufs))
        p_out = ctx.enter_context(tc.tile_pool(name="outp", bufs=bufs))

        eng_dm = nc.gpsimd if gpsimd_dm else nc.vector
        eng_fin = nc.gpsimd if gpsimd_finals else nc.vector
        eng_odma = {"scalar": nc.scalar, "sync": nc.sync, "gpsimd": nc.gpsimd}[
            out_dma
        ]

        def emit_superblock(r0):
            rgbo_t = p_in.tile([BLOCK, 4 * N], F32, tag="rgbo")
            depth_t = p_in.tile([BLOCK, N], F32, tag="depth")
            nc.sync.dma_start(
                out=rgbo_t,
                in_=rgbo_ap[r0 : r0 + SUPER].rearrange(
                    "(p t) s c -> p (t s c)", p=BLOCK
                ),
            )
            nc.sync.dma_start(
                out=depth_t,
                in_=depth_ap[r0 : r0 + SUPER].rearrange(
                    "(p t) s -> p (t s)", p=BLOCK
                ),
            )

            depth3 = depth_t.rearrange("p (t s) -> p t s", t=T)
            rgbo4 = rgbo_t.rearrange("p (t s c) -> p t s c", t=T, s=S)

            # delta / m (segmented; last sample of each ray zeroed so one
            # long scan never crosses a ray boundary with the sentinel)
            delta_t = p_mid.tile([BLOCK, N], F32, tag="delta")
            delta3 = delta_t.rearrange("p (t s) -> p t s", t=T)
            eng_dm.tensor_sub(
                delta3[:, :, 0 : S - 1], depth3[:, :, 1:S], depth3[:, :, 0 : S - 1]
            )
            m_t = p_mid.tile([BLOCK, N], F32, tag="m")
            m3 = m_t.rearrange("p (t s) -> p t s", t=T)
            eng_dm.tensor_mul(
                m3[:, :, 0 : S - 1],
                delta3[:, :, 0 : S - 1],
                rgbo4[:, :, 0 : S - 1, 3],
            )
            eng_dm.memset(m3[:, :, S - 1], 0.0)

            # one inclusive scan over the whole T*S extent
            cs_t = p_mid.tile([BLOCK, N], F32, tag="cs")
            nc.vector.tensor_tensor_scan(
                cs_t[:],
                m_t[:],
                m_t[:],
                0.0,
                mybir.AluOpType.add,
                mybir.AluOpType.bypass,
            )

            # te[j] = exp(-cs[j-1]), te[0] = 1
            te_t = p_mid.tile([BLOCK, N + 4], F32, tag="te")
            eng_fin.memset(te_t[:, 0:1], 1.0)
            nc.scalar.activation(
                te_t[:, 1 : N + 1],
                cs_t[:],
                mybir.ActivationFunctionType.Exp,
                scale=-1.0,
            )

            # per-ray un-normalizer b[t] = exp(+cs[t*S-1]); the sigmoid
            # 0.5-affine is folded into the final tensor_scalar instead.
            cs3 = cs_t.rearrange("p (t s) -> p t s", t=T)
            b_t = p_mid.tile([BLOCK, T], F32, tag="b")
            eng_fin.memset(b_t[:, 0:1], 1.0)
            nc.scalar.activation(
                b_t[:, 1:T],
                cs3[:, 0 : T - 1, S - 1],
                mybir.ActivationFunctionType.Exp,
            )

            # g[c] = tanh(rgb_c/2), all four channels in one dense-input ACT
            # op (ch3 is garbage, never read); strided ACT reads cost 2x,
            # strided writes are free.
            g4_t = p_mid.tile([BLOCK, 4 * N], BF16, tag="g4")
            if tanh4:
                nc.scalar.activation(
                    g4_t.rearrange("p (c t s) -> p t s c", c=4, t=T),
                    rgbo4,
                    mybir.ActivationFunctionType.Tanh,
                    scale=0.5,
                )
            else:
                g4v = g4_t.rearrange("p (c n) -> p c n", c=4)
                for c in range(3):
                    nc.scalar.activation(
                        g4v[:, c].rearrange("p (t s) -> p t s", t=T),
                        rgbo4[:, :, :, c],
                        mybir.ActivationFunctionType.Tanh,
                        scale=0.5,
                    )

            # w~[j] = te[j]-te[j+1] (zero at ray boundaries since m=0 there),
            # then overwrite each ray's last sample with te itself (alpha=1
            # under the FAR sentinel).
            w_t = p_mid.tile([BLOCK, N], BF16, tag="w")
            nc.vector.tensor_sub(w_t[:], te_t[:, 0:N], te_t[:, 1 : N + 1])
            te3 = te_t[:, 0:N].rearrange("p (t s) -> p t s", t=T)
            w3 = w_t.rearrange("p (t s) -> p t s", t=T)
            nc.vector.tensor_copy(w3[:, :, S - 1], te3[:, :, S - 1])

            # wg[c] = w~*g[c]  (bf16 dense -> 2x DVE mode)
            wg_t = p_mid.tile([BLOCK, 3 * N], BF16, tag="wg")
            wg3 = wg_t.rearrange("p (c n) -> p c n", c=3)
            g4f = g4_t.rearrange("p (c n) -> p c n", c=4)
            for c in range(3):
                nc.vector.tensor_mul(wg3[:, c], w_t[:], g4f[:, c])

            # segmented reduce over s -> S_c[t], layout [p, (c t)]
            s_t = p_mid.tile([BLOCK, 3 * T], F32, tag="s")
            if reduce_stt:
                scr = p_mid.tile([BLOCK, S], F32, tag="scr")
                wgseg = wg_t.rearrange("p (n s) -> p n s", s=S)
                for n in range(3 * T):
                    nc.vector.scalar_tensor_tensor(
                        out=scr[:],
                        in0=wgseg[:, n],
                        scalar=0.0,
                        in1=wgseg[:, n],
                        op0=mybir.AluOpType.bypass,
                        op1=mybir.AluOpType.bypass,
                        accum_out=s_t[:, n : n + 1],
                    )
            elif fold:
                wgf_t = p_mid.tile([BLOCK, 3 * T * (S // 2)], BF16, tag="wgf")
                wgs = wg_t.rearrange("p (n s) -> p n s", s=S)
                wgfs = wgf_t.rearrange("p (n s) -> p n s", s=S // 2)
                nc.vector.tensor_add(
                    wgfs, wgs[:, :, 0 : S // 2], wgs[:, :, S // 2 : S]
                )
                nc.vector.tensor_reduce(
                    s_t[:],
                    wgfs,
                    mybir.AxisListType.X,
                    mybir.AluOpType.add,
                )
            else:
                nc.vector.tensor_reduce(
                    s_t[:],
                    wg_t.rearrange("p (n s) -> p n s", s=S),
                    mybir.AxisListType.X,
                    mybir.AluOpType.add,
                )

            # out[t,c] = (S_c[t]*b[t])*0.5 + 0.5
            out_t = p_out.tile([BLOCK, 3 * T], F32, tag="out")
            out3 = out_t.rearrange("p (t c) -> p t c", c=3)
            s3 = s_t.rearrange("p (c t) -> p c t", c=3)
            for c in range(3):
                eng_fin.tensor_mul(out3[:, :, c], s3[:, c], b_t[:])
            out2_t = p_out.tile([BLOCK, 3 * T], F32, tag="out2")
            eng_fin.tensor_scalar(
                out2_t[:],
                out_t[:],
                0.5,
                0.5,
                mybir.AluOpType.mult,
                mybir.AluOpType.add,
            )

            eng_odma.dma_start(
                out=out_ap[r0 : r0 + SUPER].rearrange("(p t) c -> p (t c)", p=BLOCK),
                in_=out2_t[:],
            )

        for sb in range(n_super):
            emit_superblock(sb * SUPER)
    nc.compile()
    return nc


_NC_CACHE: dict = {}


def _get_nc(**kwargs):
    key = tuple(sorted(kwargs.items()))
    if key not in _NC_CACHE:
        _NC_CACHE[key] = build_nerf_bass(**kwargs)
    return _NC_CACHE[key]


def kernel(rgbo: np.ndarray, depth: np.ndarray, build_kwargs=None, **run_kwargs) -> np.ndarray:
    rgbo = np.ascontiguousarray(rgbo, dtype=np.float32)
    depth = np.ascontiguousarray(depth, dtype=np.float32)
    assert rgbo.shape == (N_RAYS, S, 4) and depth.shape == (N_RAYS, S)

    nc = _get_nc(**(build_kwargs or {}))
    in_maps = []
    for i in range(N_CORES):
        sl = slice(i * NC_RAYS, (i + 1) * NC_RAYS)
        in_maps.append({"rgbo": rgbo[sl], "depth": depth[sl]})
    res = run_bass_kernel_spmd(nc, in_maps, core_ids=list(range(N_CORES)), **run_kwargs)
    out = np.concatenate([r["out"] for r in res.results], axis=0)
    if run_kwargs:
        kernel.last_results = res  # stash for profiling harnesses
    return out


# revision 9
# speedup vs baseline: 1.1968x; 1.1968x over previous
"""NeRF volumetric alpha-compositing kernel for Trainium2 (Bass/Tile).

Full inputs:  rgbo [131072, 128, 4] f32, depth [131072, 128] f32.
Full output:  [131072, 3] f32.

Sharding: data-parallel over rays, 8 cores x 16384 rays.

Per-core algorithm, ray-per-partition layout (BLOCK=128 rays on partitions,
T rays per partition per superblock, S=128 samples each on the free dim):

  delta[s] = depth[s+1]-depth[s]; m[s] = opacity[s]*delta[s]; m[S-1] = 0
  cs       = inclusive_cumsum(m) over the whole T*S free extent (one scan;
             zeroing the last-sample sentinel keeps rays within a partition
             row from poisoning each other)
  te[j]    = exp(-cs[j-1]), te[0] = 1        (ACT Exp, scale=-1)
  b'[t]    = 0.5*exp(+cs[t*S-1])             (per-ray un-normalizer; exp
             bias=ln(1/2) folds the sigmoid->tanh affine)
  w~[j]    = te[j]-te[j+1]; w~[t,S-1] = te[t*S+S-1]   (bf16)
  g[c]     = tanh(0.5*rgb_c)                 (bf16; sigmoid(x) =
             0.5+0.5*tanh(x/2) keeps ACT on one table set with Exp)
  S_c[t]   = sum_s w~[t,s]*g[c][t,s]         (bf16 muls + segmented reduce)
  out[t,c] = S_c[t]*b'[t] + 0.5

The per-ray transmittance factor exp(+cs[ray start-1]) cancels the cross-ray
accumulation of the single long scan; sum_s w~ telescopes to 1/b exactly, so
the sigmoid "+0.5" term reduces to the constant 0.5.

Engine split: scan/w~/muls/reduce on DVE, delta/m/finals on GPSIMD,
Exp+Tanh on ACT (one table set - no ACT_TABLE_LOAD churn), input DMA on
sync, output DMA on scalar (second HWDGE ring).
"""

from contextlib import ExitStack
from math import log as _ln

import numpy as np

import concourse.bass as bass
import concourse.tile as tile
from concourse import bacc, mybir
from concourse.bass_utils import run_bass_kernel_spmd

N_RAYS = 131072
S = 128
N_CORES = 8
NC_RAYS = N_RAYS // N_CORES  # 16384 rays per core
BLOCK = 128                  # rays per partition-block
F32 = mybir.dt.float32
BF16 = mybir.dt.bfloat16
LN_HALF = _ln(0.5)


def build_nerf_bass(
    n_rays: int = NC_RAYS,
    t_blocks: int = 8,
    bufs: int = 2,
    gpsimd_dm: bool = True,
    gpsimd_finals: bool = True,
    tanh4: bool = True,
    fold: bool = False,
    reduce_stt: bool = False,
    out_dma: str = "scalar",
    g_psum: bool = False,
    cs_psum: bool = False,
    wg_psum: bool = False,
) -> bass.Bass:
    T = t_blocks
    SUPER = BLOCK * T
    assert n_rays % SUPER == 0
    n_super = n_rays // SUPER
    N = S * T  # free extent per partition

    nc = bacc.Bacc("TRN2", target_bir_lowering=False, debug=False)
    rgbo_h = nc.declare_dram_parameter("rgbo", [n_rays, S, 4], F32, isOutput=False)
    depth_h = nc.declare_dram_parameter("depth", [n_rays, S], F32, isOutput=False)
    out_h = nc.declare_dram_parameter("out", [n_rays, 3], F32, isOutput=True)

    rgbo_ap = rgbo_h.ap()
    depth_ap = depth_h.ap()
    out_ap = out_h.ap()

    with ExitStack() as ctx:
        tc = ctx.enter_context(tile.TileContext(nc))
        p_in = ctx.enter_context(tc.tile_pool(name="inp", bufs=bufs))
        p_mid = ctx.enter_context(tc.tile_pool(name="mid", bufs=bufs))
        p_out = ctx.enter_context(tc.tile_pool(name="outp", bufs=bufs))
        p_ps = (
            ctx.enter_context(tc.tile_pool(name="ps", bufs=bufs, space="PSUM"))
            if (g_psum or cs_psum or wg_psum)
            else None
        )

        eng_dm = nc.gpsimd if gpsimd_dm else nc.vector
        eng_fin = nc.gpsimd if gpsimd_finals else nc.vector
        eng_odma = {"scalar": nc.scalar, "sync": nc.sync, "gpsimd": nc.gpsimd}[
            out_dma
        ]

        def emit_superblock(r0):
            rgbo_t = p_in.tile([BLOCK, 4 * N], F32, tag="rgbo")
            depth_t = p_in.tile([BLOCK, N], F32, tag="depth")
            nc.sync.dma_start(
                out=rgbo_t,
                in_=rgbo_ap[r0 : r0 + SUPER].rearrange(
                    "(p t) s c -> p (t s c)", p=BLOCK
                ),
            )
            nc.sync.dma_start(
                out=depth_t,
                in_=depth_ap[r0 : r0 + SUPER].rearrange(
                    "(p t) s -> p (t s)", p=BLOCK
                ),
            )

            depth3 = depth_t.rearrange("p (t s) -> p t s", t=T)
            rgbo4 = rgbo_t.rearrange("p (t s c) -> p t s c", t=T, s=S)

            # delta / m (segmented; last sample of each ray zeroed so one
            # long scan never crosses a ray boundary with the sentinel)
            delta_t = p_mid.tile([BLOCK, N], F32, tag="delta")
            delta3 = delta_t.rearrange("p (t s) -> p t s", t=T)
            eng_dm.tensor_sub(
                delta3[:, :, 0 : S - 1], depth3[:, :, 1:S], depth3[:, :, 0 : S - 1]
            )
            m_t = p_mid.tile([BLOCK, N], F32, tag="m")
            m3 = m_t.rearrange("p (t s) -> p t s", t=T)
            eng_dm.tensor_mul(
                m3[:, :, 0 : S - 1],
                delta3[:, :, 0 : S - 1],
                rgbo4[:, :, 0 : S - 1, 3],
            )
            eng_dm.memset(m3[:, :, S - 1], 0.0)

            # one inclusive scan over the whole T*S extent
            cs_pool = p_ps if cs_psum else p_mid
            cs_t = cs_pool.tile([BLOCK, N], F32, tag="cs")
            nc.vector.tensor_tensor_scan(
                cs_t[:],
                m_t[:],
                m_t[:],
                0.0,
                mybir.AluOpType.add,
                mybir.AluOpType.bypass,
            )

            # te[j] = exp(-cs[j-1]), te[0] = 1
            te_t = p_mid.tile([BLOCK, N + 4], F32, tag="te")
            eng_fin.memset(te_t[:, 0:1], 1.0)
            nc.scalar.activation(
                te_t[:, 1 : N + 1],
                cs_t[:],
                mybir.ActivationFunctionType.Exp,
                scale=-1.0,
            )

            # per-ray un-normalizer b[t] = exp(+cs[t*S-1]); the sigmoid
            # 0.5-affine is folded into the final tensor_scalar instead.
            cs3 = cs_t.rearrange("p (t s) -> p t s", t=T)
            b_t = p_mid.tile([BLOCK, T], F32, tag="b")
            eng_fin.memset(b_t[:, 0:1], 1.0)
            nc.scalar.activation(
                b_t[:, 1:T],
                cs3[:, 0 : T - 1, S - 1],
                mybir.ActivationFunctionType.Exp,
            )

            # g[c] = tanh(rgb_c/2), all four channels in one dense-input ACT
            # op (ch3 is garbage, never read); strided ACT reads cost 2x,
            # strided writes are free.
            g4_t = (p_ps if g_psum else p_mid).tile([BLOCK, 4 * N], BF16, tag="g4")
            if tanh4:
                nc.scalar.activation(
                    g4_t.rearrange("p (c t s) -> p t s c", c=4, t=T),
                    rgbo4,
                    mybir.ActivationFunctionType.Tanh,
                    scale=0.5,
                )
            else:
                g4v = g4_t.rearrange("p (c n) -> p c n", c=4)
                for c in range(3):
                    nc.scalar.activation(
                        g4v[:, c].rearrange("p (t s) -> p t s", t=T),
                        rgbo4[:, :, :, c],
                        mybir.ActivationFunctionType.Tanh,
                        scale=0.5,
                    )

            # w~[j] = te[j]-te[j+1] (zero at ray boundaries since m=0 there),
            # then overwrite each ray's last sample with te itself (alpha=1
            # under the FAR sentinel).
            w_t = p_mid.tile([BLOCK, N], BF16, tag="w")
            nc.vector.tensor_sub(w_t[:], te_t[:, 0:N], te_t[:, 1 : N + 1])
            te3 = te_t[:, 0:N].rearrange("p (t s) -> p t s", t=T)
            w3 = w_t.rearrange("p (t s) -> p t s", t=T)
            nc.vector.tensor_copy(w3[:, :, S - 1], te3[:, :, S - 1])

            # wg[c] = w~*g[c]  (bf16 dense -> 2x DVE mode)
            wg_t = (p_ps if wg_psum else p_mid).tile([BLOCK, 3 * N], BF16, tag="wg")
            wg3 = wg_t.rearrange("p (c n) -> p c n", c=3)
            g4f = g4_t.rearrange("p (c n) -> p c n", c=4)
            for c in range(3):
                nc.vector.tensor_mul(wg3[:, c], w_t[:], g4f[:, c])

            # segmented reduce over s -> S_c[t], layout [p, (c t)]
            s_t = p_mid.tile([BLOCK, 3 * T], F32, tag="s")
            if reduce_stt:
                scr = p_mid.tile([BLOCK, S], F32, tag="scr")
                wgseg = wg_t.rearrange("p (n s) -> p n s", s=S)
                for n in range(3 * T):
                    nc.vector.scalar_tensor_tensor(
                        out=scr[:],
                        in0=wgseg[:, n],
                        scalar=0.0,
                        in1=wgseg[:, n],
                        op0=mybir.AluOpType.bypass,
                        op1=mybir.AluOpType.bypass,
                        accum_out=s_t[:, n : n + 1],
                    )
            elif fold:
                wgf_t = p_mid.tile([BLOCK, 3 * T * (S // 2)], BF16, tag="wgf")
                wgs = wg_t.rearrange("p (n s) -> p n s", s=S)
                wgfs = wgf_t.rearrange("p (n s) -> p n s", s=S // 2)
                nc.vector.tensor_add(
                    wgfs, wgs[:, :, 0 : S // 2], wgs[:, :, S // 2 : S]
                )
                nc.vector.tensor_reduce(
                    s_t[:],
                    wgfs,
                    mybir.AxisListType.X,
                    mybir.AluOpType.add,
                )
            else:
                nc.vector.tensor_reduce(
                    s_t[:],
                    wg_t.rearrange("p (n s) -> p n s", s=S),
                    mybir.AxisListType.X,
                    mybir.AluOpType.add,
                )

            # out[t,c] = (S_c[t]*b[t])*0.5 + 0.5
            out_t = p_out.tile([BLOCK, 3 * T], F32, tag="out")
            out3 = out_t.rearrange("p (t c) -> p t c", c=3)
            s3 = s_t.rearrange("p (c t) -> p c t", c=3)
            for c in range(3):
                eng_fin.tensor_mul(out3[:, :, c], s3[:, c], b_t[:])
            out2_t = p_out.tile([BLOCK, 3 * T], F32, tag="out2")
            eng_fin.tensor_scalar(
                out2_t[:],
                out_t[:],
                0.5,
                0.5,
                mybir.AluOpType.mult,
                mybir.AluOpType.add,
            )

            eng_odma.dma_start(
                out=out_ap[r0 : r0 + SUPER].rearrange("(p t) c -> p (t c)", p=BLOCK),
                in_=out2_t[:],
            )

        for sb in range(n_super):
            emit_superblock(sb * SUPER)
    nc.compile()
    return nc


_NC_CACHE: dict = {}


def _get_nc(**kwargs):
    key = tuple(sorted(kwargs.items()))
    if key not in _NC_CACHE:
        _NC_CACHE[key] = build_nerf_bass(**kwargs)
    return _NC_CACHE[key]


def kernel(rgbo: np.ndarray, depth: np.ndarray, build_kwargs=None, **run_kwargs) -> np.ndarray:
    rgbo = np.ascontiguousarray(rgbo, dtype=np.float32)
    depth = np.ascontiguousarray(depth, dtype=np.float32)
    assert rgbo.shape == (N_RAYS, S, 4) and depth.shape == (N_RAYS, S)

    nc = _get_nc(**(build_kwargs or {}))
    in_maps = []
    for i in range(N_CORES):
        sl = slice(i * NC_RAYS, (i + 1) * NC_RAYS)
        in_maps.append({"rgbo": rgbo[sl], "depth": depth[sl]})
    res = run_bass_kernel_spmd(nc, in_maps, core_ids=list(range(N_CORES)), **run_kwargs)
    out = np.concatenate([r["out"] for r in res.results], axis=0)
    if run_kwargs:
        kernel.last_results = res  # stash for profiling harnesses
    return out


# revision 14
# speedup vs baseline: 1.3393x; 1.1191x over previous
"""NeRF volumetric alpha-compositing kernel for Trainium2 (Bass/Tile).

Full inputs:  rgbo [131072, 128, 4] f32, depth [131072, 128] f32.
Full output:  [131072, 3] f32.

Sharding: data-parallel over rays, 8 cores x 16384 rays.

Per-core algorithm, ray-per-partition layout (BLOCK=128 rays on partitions,
T rays per partition per superblock, S=128 samples each on the free dim):

  delta[s] = depth[s+1]-depth[s]; m[s] = opacity[s]*delta[s]; m[S-1] = 0
  cs       = inclusive_cumsum(m) over the whole T*S free extent (one scan;
             zeroing the last-sample sentinel keeps rays within a partition
             row from poisoning each other)
  te[j]    = exp(-cs[j-1]), te[0] = 1        (ACT Exp, scale=-1)
  b'[t]    = 0.5*exp(+cs[t*S-1])             (per-ray un-normalizer; exp
             bias=ln(1/2) folds the sigmoid->tanh affine)
  w~[j]    = te[j]-te[j+1]; w~[t,S-1] = te[t*S+S-1]   (bf16)
  g[c]     = tanh(0.5*rgb_c)                 (bf16; sigmoid(x) =
             0.5+0.5*tanh(x/2) keeps ACT on one table set with Exp)
  S_c[t]   = sum_s w~[t,s]*g[c][t,s]         (bf16 muls + segmented reduce)
  out[t,c] = S_c[t]*b'[t] + 0.5

The per-ray transmittance factor exp(+cs[ray start-1]) cancels the cross-ray
accumulation of the single long scan; sum_s w~ telescopes to 1/b exactly, so
the sigmoid "+0.5" term reduces to the constant 0.5.

Engine split: scan/w~/muls/reduce on DVE, delta/m/finals on GPSIMD,
Exp+Tanh on ACT (one table set - no ACT_TABLE_LOAD churn), input DMA on
sync, output DMA on scalar (second HWDGE ring).
"""

from contextlib import ExitStack
from math import log as _ln

import numpy as np

import concourse.bass as bass
import concourse.tile as tile
from concourse import bacc, mybir
from concourse.bass_utils import run_bass_kernel_spmd

N_RAYS = 131072
S = 128
N_CORES = 8
NC_RAYS = N_RAYS // N_CORES  # 16384 rays per core
BLOCK = 128                  # rays per partition-block
F32 = mybir.dt.float32
BF16 = mybir.dt.bfloat16
LN_HALF = _ln(0.5)


def build_nerf_bass(
    n_rays: int = NC_RAYS,
    t_blocks: int = 8,
    bufs: int = 2,
    gpsimd_dm: bool = True,
    gpsimd_finals: bool = True,
    tanh4: bool = True,
    fold: int = 0,
    reduce_stt: bool = False,
    out_dma: str = "scalar",
    g_psum: bool = False,
    cs_psum: bool = False,
    wg_psum: bool = False,
    scan_c1: bool = False,
) -> bass.Bass:
    T = t_blocks
    SUPER = BLOCK * T
    assert n_rays % SUPER == 0
    n_super = n_rays // SUPER
    N = S * T  # free extent per partition

    nc = bacc.Bacc("TRN2", target_bir_lowering=False, debug=False)
    rgbo_h = nc.declare_dram_parameter("rgbo", [n_rays, S, 4], F32, isOutput=False)
    depth_h = nc.declare_dram_parameter("depth", [n_rays, S], F32, isOutput=False)
    out_h = nc.declare_dram_parameter("out", [n_rays, 3], F32, isOutput=True)

    rgbo_ap = rgbo_h.ap()
    depth_ap = depth_h.ap()
    out_ap = out_h.ap()

    with ExitStack() as ctx:
        tc = ctx.enter_context(tile.TileContext(nc))
        p_in = ctx.enter_context(tc.tile_pool(name="inp", bufs=bufs))
        p_mid = ctx.enter_context(tc.tile_pool(name="mid", bufs=bufs))
        p_out = ctx.enter_context(tc.tile_pool(name="outp", bufs=bufs))
        p_ps = (
            ctx.enter_context(tc.tile_pool(name="ps", bufs=1, space="PSUM"))
            if (g_psum or cs_psum or wg_psum)
            else None
        )

        eng_dm = nc.gpsimd if gpsimd_dm else nc.vector
        eng_fin = nc.gpsimd if gpsimd_finals else nc.vector
        eng_odma = {"scalar": nc.scalar, "sync": nc.sync, "gpsimd": nc.gpsimd}[
            out_dma
        ]

        def emit_superblock(r0):
            rgbo_t = p_in.tile([BLOCK, 4 * N], F32, tag="rgbo")
            depth_t = p_in.tile([BLOCK, N], F32, tag="depth")
            nc.sync.dma_start(
                out=rgbo_t,
                in_=rgbo_ap[r0 : r0 + SUPER].rearrange(
                    "(p t) s c -> p (t s c)", p=BLOCK
                ),
            )
            nc.sync.dma_start(
                out=depth_t,
                in_=depth_ap[r0 : r0 + SUPER].rearrange(
                    "(p t) s -> p (t s)", p=BLOCK
                ),
            )

            depth3 = depth_t.rearrange("p (t s) -> p t s", t=T)
            rgbo4 = rgbo_t.rearrange("p (t s c) -> p t s c", t=T, s=S)

            # delta / m (segmented; last sample of each ray zeroed so one
            # long scan never crosses a ray boundary with the sentinel)
            delta_t = p_mid.tile([BLOCK, N], F32, tag="delta")
            delta3 = delta_t.rearrange("p (t s) -> p t s", t=T)
            eng_dm.tensor_sub(
                delta3[:, :, 0 : S - 1], depth3[:, :, 1:S], depth3[:, :, 0 : S - 1]
            )
            m_t = p_mid.tile([BLOCK, N], F32, tag="m")
            m3 = m_t.rearrange("p (t s) -> p t s", t=T)
            eng_dm.tensor_mul(
                m3[:, :, 0 : S - 1],
                delta3[:, :, 0 : S - 1],
                rgbo4[:, :, 0 : S - 1, 3],
            )
            eng_dm.memset(m3[:, :, S - 1], 0.0)

            # one inclusive scan over the whole T*S extent
            cs_pool = p_ps if cs_psum else p_mid
            cs_t = cs_pool.tile([BLOCK, N], F32, tag="cs")
            scan_d1 = (
                nc.const_aps.tensor(0.0, (BLOCK, N)) if scan_c1 else m_t[:]
            )
            nc.vector.tensor_tensor_scan(
                cs_t[:],
                m_t[:],
                scan_d1,
                0.0,
                mybir.AluOpType.add,
                mybir.AluOpType.bypass,
            )

            # te[j] = exp(-cs[j-1]), te[0] = 1
            te_t = p_mid.tile([BLOCK, N + 4], F32, tag="te")
            eng_fin.memset(te_t[:, 0:1], 1.0)
            nc.scalar.activation(
                te_t[:, 1 : N + 1],
                cs_t[:],
                mybir.ActivationFunctionType.Exp,
                scale=-1.0,
            )

            # per-ray un-normalizer b[t] = exp(+cs[t*S-1]); the sigmoid
            # 0.5-affine is folded into the final tensor_scalar instead.
            cs3 = cs_t.rearrange("p (t s) -> p t s", t=T)
            b_t = p_mid.tile([BLOCK, T], F32, tag="b")
            eng_fin.memset(b_t[:, 0:1], 1.0)
            nc.scalar.activation(
                b_t[:, 1:T],
                cs3[:, 0 : T - 1, S - 1],
                mybir.ActivationFunctionType.Exp,
            )

            # g = tanh(rgb/2). tanh4: one dense->dense ACT op over all 4
            # interleaved channels (ch3 is garbage, never read) - both APs
            # flat, so ACT runs at 1 elem/cycle. The consumer reads g
            # strided instead (DVE 1x). Alternative: 3 per-channel ops with
            # strided reads (2x ACT cost) but dense bf16 outputs (2x DVE).
            g4_t = (p_ps if g_psum else p_mid).tile([BLOCK, 4 * N], BF16, tag="g4")
            if tanh4:
                nc.scalar.activation(
                    g4_t[:],
                    rgbo_t[:],
                    mybir.ActivationFunctionType.Tanh,
                    scale=0.5,
                )
                g_views = [
                    g4_t.rearrange("p (j c) -> p c j", c=4)[:, c] for c in range(3)
                ]
            else:
                g4v = g4_t.rearrange("p (c n) -> p c n", c=4)
                for c in range(3):
                    nc.scalar.activation(
                        g4v[:, c].rearrange("p (t s) -> p t s", t=T),
                        rgbo4[:, :, :, c],
                        mybir.ActivationFunctionType.Tanh,
                        scale=0.5,
                    )
                g_views = [g4v[:, c] for c in range(3)]

            # w~[j] = te[j]-te[j+1] (zero at ray boundaries since m=0 there),
            # then overwrite each ray's last sample with te itself (alpha=1
            # under the FAR sentinel).
            w_t = p_mid.tile([BLOCK, N], BF16, tag="w")
            nc.vector.tensor_sub(w_t[:], te_t[:, 0:N], te_t[:, 1 : N + 1])
            te3 = te_t[:, 0:N].rearrange("p (t s) -> p t s", t=T)
            w3 = w_t.rearrange("p (t s) -> p t s", t=T)
            nc.vector.tensor_copy(w3[:, :, S - 1], te3[:, :, S - 1])

            # wg[c] = w~*g[c]
            wg_t = (p_ps if wg_psum else p_mid).tile([BLOCK, 3 * N], BF16, tag="wg")
            wg3 = wg_t.rearrange("p (c n) -> p c n", c=3)
            for c in range(3):
                nc.vector.tensor_mul(wg3[:, c], w_t[:], g_views[c])

            # segmented reduce over s -> S_c[t], layout [p, (c t)], with
            # `fold` levels of bf16 pairwise adds (2x mode) shrinking the
            # 1x-only tensor_reduce input first.
            s_t = p_mid.tile([BLOCK, 3 * T], F32, tag="s")
            red_in, seg = wg_t, S
            for lvl in range(fold):
                half = seg // 2
                f_t = p_mid.tile([BLOCK, 3 * T * half], BF16, tag=f"wgf{lvl}")
                nc.vector.tensor_add(
                    f_t.rearrange("p (n s) -> p n s", s=half),
                    red_in.rearrange("p (n s) -> p n s", s=seg)[:, :, 0:half],
                    red_in.rearrange("p (n s) -> p n s", s=seg)[:, :, half:seg],
                )
                red_in, seg = f_t, half
            nc.vector.tensor_reduce(
                s_t[:],
                red_in.rearrange("p (n s) -> p n s", s=seg),
                mybir.AxisListType.X,
                mybir.AluOpType.add,
            )

            # out[t,c] = (S_c[t]*b[t])*0.5 + 0.5
            out_t = p_out.tile([BLOCK, 3 * T], F32, tag="out")
            out3 = out_t.rearrange("p (t c) -> p t c", c=3)
            s3 = s_t.rearrange("p (c t) -> p c t", c=3)
            for c in range(3):
                eng_fin.tensor_mul(out3[:, :, c], s3[:, c], b_t[:])
            out2_t = p_out.tile([BLOCK, 3 * T], F32, tag="out2")
            eng_fin.tensor_scalar(
                out2_t[:],
                out_t[:],
                0.5,
                0.5,
                mybir.AluOpType.mult,
                mybir.AluOpType.add,
            )

            eng_odma.dma_start(
                out=out_ap[r0 : r0 + SUPER].rearrange("(p t) c -> p (t c)", p=BLOCK),
                in_=out2_t[:],
            )

        for sb in range(n_super):
            emit_superblock(sb * SUPER)
    nc.compile()
    return nc


_NC_CACHE: dict = {}


def _get_nc(**kwargs):
    key = tuple(sorted(kwargs.items()))
    if key not in _NC_CACHE:
        _NC_CACHE[key] = build_nerf_bass(**kwargs)
    return _NC_CACHE[key]


def kernel(rgbo: np.ndarray, depth: np.ndarray, build_kwargs=None, **run_kwargs) -> np.ndarray:
    rgbo = np.ascontiguousarray(rgbo, dtype=np.float32)
    depth = np.ascontiguousarray(depth, dtype=np.float32)
    assert rgbo.shape == (N_RAYS, S, 4) and depth.shape == (N_RAYS, S)

    nc = _get_nc(**(build_kwargs or {}))
    in_maps = []
    for i in range(N_CORES):
        sl = slice(i * NC_RAYS, (i + 1) * NC_RAYS)
        in_maps.append({"rgbo": rgbo[sl], "depth": depth[sl]})
    res = run_bass_kernel_spmd(nc, in_maps, core_ids=list(range(N_CORES)), **run_kwargs)
    out = np.concatenate([r["out"] for r in res.results], axis=0)
    if run_kwargs:
        kernel.last_results = res  # stash for profiling harnesses
    return out


# revision 16
# speedup vs baseline: 1.3468x; 1.0056x over previous
"""NeRF volumetric alpha-compositing kernel for Trainium2 (Bass/Tile).

Full inputs:  rgbo [131072, 128, 4] f32, depth [131072, 128] f32.
Full output:  [131072, 3] f32.

Sharding: data-parallel over rays, 8 cores x 16384 rays.

Per-core algorithm, ray-per-partition layout (BLOCK=128 rays on partitions,
T rays per partition per superblock, S=128 samples each on the free dim):

  delta[s] = depth[s+1]-depth[s]; m[s] = opacity[s]*delta[s]; m[S-1] = 0
  cs       = inclusive_cumsum(m) over the whole T*S free extent (one scan;
             zeroing the last-sample sentinel keeps rays within a partition
             row from poisoning each other)
  te[j]    = exp(-cs[j-1]), te[0] = 1        (ACT Exp, scale=-1)
  b'[t]    = 0.5*exp(+cs[t*S-1])             (per-ray un-normalizer; exp
             bias=ln(1/2) folds the sigmoid->tanh affine)
  w~[j]    = te[j]-te[j+1]; w~[t,S-1] = te[t*S+S-1]   (bf16)
  g[c]     = tanh(0.5*rgb_c)                 (bf16; sigmoid(x) =
             0.5+0.5*tanh(x/2) keeps ACT on one table set with Exp)
  S_c[t]   = sum_s w~[t,s]*g[c][t,s]         (bf16 muls + segmented reduce)
  out[t,c] = S_c[t]*b'[t] + 0.5

The per-ray transmittance factor exp(+cs[ray start-1]) cancels the cross-ray
accumulation of the single long scan; sum_s w~ telescopes to 1/b exactly, so
the sigmoid "+0.5" term reduces to the constant 0.5.

Engine split: scan/w~/muls/reduce on DVE, delta/m/finals on GPSIMD,
Exp+Tanh on ACT (one table set - no ACT_TABLE_LOAD churn), input DMA on
sync, output DMA on scalar (second HWDGE ring).
"""

from contextlib import ExitStack
from math import log as _ln

import numpy as np

import concourse.bass as bass
import concourse.tile as tile
from concourse import bacc, mybir
from concourse.bass_utils import run_bass_kernel_spmd

N_RAYS = 131072
S = 128
N_CORES = 8
NC_RAYS = N_RAYS // N_CORES  # 16384 rays per core
BLOCK = 128                  # rays per partition-block
F32 = mybir.dt.float32
BF16 = mybir.dt.bfloat16
LN_HALF = _ln(0.5)


def build_nerf_bass(
    n_rays: int = NC_RAYS,
    t_blocks: int = 8,
    bufs: int = 2,
    mid_bufs: int = 0,
    gpsimd_dm: bool = True,
    gpsimd_finals: bool = True,
    tanh4: bool = True,
    fold: int = 0,
    reduce_stt: bool = False,
    out_dma: str = "scalar",
    g_psum: bool = False,
    cs_psum: bool = False,
    wg_psum: bool = False,
    scan_c1: bool = False,
) -> bass.Bass:
    T = t_blocks
    SUPER = BLOCK * T
    assert n_rays % SUPER == 0
    n_super = n_rays // SUPER
    N = S * T  # free extent per partition

    nc = bacc.Bacc("TRN2", target_bir_lowering=False, debug=False)
    rgbo_h = nc.declare_dram_parameter("rgbo", [n_rays, S, 4], F32, isOutput=False)
    depth_h = nc.declare_dram_parameter("depth", [n_rays, S], F32, isOutput=False)
    out_h = nc.declare_dram_parameter("out", [n_rays, 3], F32, isOutput=True)

    rgbo_ap = rgbo_h.ap()
    depth_ap = depth_h.ap()
    out_ap = out_h.ap()

    with ExitStack() as ctx:
        tc = ctx.enter_context(tile.TileContext(nc))
        p_in = ctx.enter_context(tc.tile_pool(name="inp", bufs=bufs))
        p_mid = ctx.enter_context(tc.tile_pool(name="mid", bufs=mid_bufs or bufs))
        p_out = ctx.enter_context(tc.tile_pool(name="outp", bufs=bufs))
        p_ps = (
            ctx.enter_context(tc.tile_pool(name="ps", bufs=1, space="PSUM"))
            if (g_psum or cs_psum or wg_psum)
            else None
        )

        eng_dm = nc.gpsimd if gpsimd_dm else nc.vector
        eng_fin = nc.gpsimd if gpsimd_finals else nc.vector
        eng_odma = {"scalar": nc.scalar, "sync": nc.sync, "gpsimd": nc.gpsimd}[
            out_dma
        ]

        def emit_superblock(r0):
            rgbo_t = p_in.tile([BLOCK, 4 * N], F32, tag="rgbo")
            depth_t = p_in.tile([BLOCK, N], F32, tag="depth")
            nc.sync.dma_start(
                out=rgbo_t,
                in_=rgbo_ap[r0 : r0 + SUPER].rearrange(
                    "(p t) s c -> p (t s c)", p=BLOCK
                ),
            )
            nc.sync.dma_start(
                out=depth_t,
                in_=depth_ap[r0 : r0 + SUPER].rearrange(
                    "(p t) s -> p (t s)", p=BLOCK
                ),
            )

            depth3 = depth_t.rearrange("p (t s) -> p t s", t=T)
            rgbo4 = rgbo_t.rearrange("p (t s c) -> p t s c", t=T, s=S)

            # delta / m (segmented; last sample of each ray zeroed so one
            # long scan never crosses a ray boundary with the sentinel)
            delta_t = p_mid.tile([BLOCK, N], F32, tag="delta")
            delta3 = delta_t.rearrange("p (t s) -> p t s", t=T)
            eng_dm.tensor_sub(
                delta3[:, :, 0 : S - 1], depth3[:, :, 1:S], depth3[:, :, 0 : S - 1]
            )
            m_t = p_mid.tile([BLOCK, N], F32, tag="m")
            m3 = m_t.rearrange("p (t s) -> p t s", t=T)
            eng_dm.tensor_mul(
                m3[:, :, 0 : S - 1],
                delta3[:, :, 0 : S - 1],
                rgbo4[:, :, 0 : S - 1, 3],
            )
            eng_dm.memset(m3[:, :, S - 1], 0.0)

            # one inclusive scan over the whole T*S extent
            cs_pool = p_ps if cs_psum else p_mid
            cs_t = cs_pool.tile([BLOCK, N], F32, tag="cs")
            scan_d1 = (
                nc.const_aps.tensor(0.0, (BLOCK, N)) if scan_c1 else m_t[:]
            )
            nc.vector.tensor_tensor_scan(
                cs_t[:],
                m_t[:],
                scan_d1,
                0.0,
                mybir.AluOpType.add,
                mybir.AluOpType.bypass,
            )

            # te[j] = exp(-cs[j-1]), te[0] = 1
            te_t = p_mid.tile([BLOCK, N + 4], F32, tag="te")
            eng_fin.memset(te_t[:, 0:1], 1.0)
            nc.scalar.activation(
                te_t[:, 1 : N + 1],
                cs_t[:],
                mybir.ActivationFunctionType.Exp,
                scale=-1.0,
            )

            # per-ray un-normalizer b[t] = exp(+cs[t*S-1]); the sigmoid
            # 0.5-affine is folded into the final tensor_scalar instead.
            cs3 = cs_t.rearrange("p (t s) -> p t s", t=T)
            b_t = p_mid.tile([BLOCK, T], F32, tag="b")
            eng_fin.memset(b_t[:, 0:1], 1.0)
            nc.scalar.activation(
                b_t[:, 1:T],
                cs3[:, 0 : T - 1, S - 1],
                mybir.ActivationFunctionType.Exp,
            )

            # g = tanh(rgb/2). tanh4: one dense->dense ACT op over all 4
            # interleaved channels (ch3 is garbage, never read) - both APs
            # flat, so ACT runs at 1 elem/cycle. The consumer reads g
            # strided instead (DVE 1x). Alternative: 3 per-channel ops with
            # strided reads (2x ACT cost) but dense bf16 outputs (2x DVE).
            g4_t = (p_ps if g_psum else p_mid).tile([BLOCK, 4 * N], BF16, tag="g4")
            if tanh4:
                nc.scalar.activation(
                    g4_t[:],
                    rgbo_t[:],
                    mybir.ActivationFunctionType.Tanh,
                    scale=0.5,
                )
                g_views = [
                    g4_t.rearrange("p (j c) -> p c j", c=4)[:, c] for c in range(3)
                ]
            else:
                g4v = g4_t.rearrange("p (c n) -> p c n", c=4)
                for c in range(3):
                    nc.scalar.activation(
                        g4v[:, c].rearrange("p (t s) -> p t s", t=T),
                        rgbo4[:, :, :, c],
                        mybir.ActivationFunctionType.Tanh,
                        scale=0.5,
                    )
                g_views = [g4v[:, c] for c in range(3)]

            # w~[j] = te[j]-te[j+1] (zero at ray boundaries since m=0 there),
            # then overwrite each ray's last sample with te itself (alpha=1
            # under the FAR sentinel).
            w_t = p_mid.tile([BLOCK, N], BF16, tag="w")
            nc.vector.tensor_sub(w_t[:], te_t[:, 0:N], te_t[:, 1 : N + 1])
            te3 = te_t[:, 0:N].rearrange("p (t s) -> p t s", t=T)
            w3 = w_t.rearrange("p (t s) -> p t s", t=T)
            nc.vector.tensor_copy(w3[:, :, S - 1], te3[:, :, S - 1])

            # wg[c] = w~*g[c]
            wg_t = (p_ps if wg_psum else p_mid).tile([BLOCK, 3 * N], BF16, tag="wg")
            wg3 = wg_t.rearrange("p (c n) -> p c n", c=3)
            for c in range(3):
                nc.vector.tensor_mul(wg3[:, c], w_t[:], g_views[c])

            # segmented reduce over s -> S_c[t], layout [p, (c t)], with
            # `fold` levels of bf16 pairwise adds (2x mode) shrinking the
            # 1x-only tensor_reduce input first.
            s_t = p_mid.tile([BLOCK, 3 * T], F32, tag="s")
            red_in, seg = wg_t, S
            for lvl in range(fold):
                half = seg // 2
                f_t = p_mid.tile([BLOCK, 3 * T * half], BF16, tag=f"wgf{lvl}")
                nc.vector.tensor_add(
                    f_t.rearrange("p (n s) -> p n s", s=half),
                    red_in.rearrange("p (n s) -> p n s", s=seg)[:, :, 0:half],
                    red_in.rearrange("p (n s) -> p n s", s=seg)[:, :, half:seg],
                )
                red_in, seg = f_t, half
            nc.vector.tensor_reduce(
                s_t[:],
                red_in.rearrange("p (n s) -> p n s", s=seg),
                mybir.AxisListType.X,
                mybir.AluOpType.add,
            )

            # out[t,c] = (S_c[t]*b[t])*0.5 + 0.5
            out_t = p_out.tile([BLOCK, 3 * T], F32, tag="out")
            out3 = out_t.rearrange("p (t c) -> p t c", c=3)
            s3 = s_t.rearrange("p (c t) -> p c t", c=3)
            for c in range(3):
                eng_fin.tensor_mul(out3[:, :, c], s3[:, c], b_t[:])
            out2_t = p_out.tile([BLOCK, 3 * T], F32, tag="out2")
            eng_fin.tensor_scalar(
                out2_t[:],
                out_t[:],
                0.5,
                0.5,
                mybir.AluOpType.mult,
                mybir.AluOpType.add,
            )

            eng_odma.dma_start(
                out=out_ap[r0 : r0 + SUPER].rearrange("(p t) c -> p (t c)", p=BLOCK),
                in_=out2_t[:],
            )

        for sb in range(n_super):
            emit_superblock(sb * SUPER)
    nc.compile()
    return nc


_NC_CACHE: dict = {}


def _get_nc(**kwargs):
    key = tuple(sorted(kwargs.items()))
    if key not in _NC_CACHE:
        _NC_CACHE[key] = build_nerf_bass(**kwargs)
    return _NC_CACHE[key]


def kernel(rgbo: np.ndarray, depth: np.ndarray, build_kwargs=None, **run_kwargs) -> np.ndarray:
    rgbo = np.ascontiguousarray(rgbo, dtype=np.float32)
    depth = np.ascontiguousarray(depth, dtype=np.float32)
    assert rgbo.shape == (N_RAYS, S, 4) and depth.shape == (N_RAYS, S)

    nc = _get_nc(**(build_kwargs or {}))
    in_maps = []
    for i in range(N_CORES):
        sl = slice(i * NC_RAYS, (i + 1) * NC_RAYS)
        in_maps.append({"rgbo": rgbo[sl], "depth": depth[sl]})
    res = run_bass_kernel_spmd(nc, in_maps, core_ids=list(range(N_CORES)), **run_kwargs)
    out = np.concatenate([r["out"] for r in res.results], axis=0)
    if run_kwargs:
        kernel.last_results = res  # stash for profiling harnesses
    return out


# revision 24
# speedup vs baseline: 1.4846x; 1.1024x over previous
"""NeRF volumetric alpha-compositing kernel for Trainium2 (Bass/Tile).

Full inputs:  rgbo [131072, 128, 4] f32, depth [131072, 128] f32.
Full output:  [131072, 3] f32.

Sharding: data-parallel over rays, 8 cores x 16384 rays.

Per-core algorithm, ray-per-partition layout (BLOCK=128 rays on partitions,
T rays per partition per superblock, S=128 samples each on the free dim):

  delta[s] = depth[s+1]-depth[s]; m[s] = opacity[s]*delta[s]; m[S-1] = 0
  cs       = inclusive_cumsum(m) over the whole T*S free extent (one scan;
             zeroing the last-sample sentinel keeps rays within a partition
             row from poisoning each other)
  te[j]    = exp(-cs[j-1]), te[0] = 1        (ACT Exp, scale=-1)
  b'[t]    = 0.5*exp(+cs[t*S-1])             (per-ray un-normalizer; exp
             bias=ln(1/2) folds the sigmoid->tanh affine)
  w~[j]    = te[j]-te[j+1]; w~[t,S-1] = te[t*S+S-1]   (bf16)
  g[c]     = tanh(0.5*rgb_c)                 (bf16; sigmoid(x) =
             0.5+0.5*tanh(x/2) keeps ACT on one table set with Exp)
  S_c[t]   = sum_s w~[t,s]*g[c][t,s]         (bf16 muls + segmented reduce)
  out[t,c] = S_c[t]*b'[t] + 0.5

The per-ray transmittance factor exp(+cs[ray start-1]) cancels the cross-ray
accumulation of the single long scan; sum_s w~ telescopes to 1/b exactly, so
the sigmoid "+0.5" term reduces to the constant 0.5.

Engine split: scan/w~/muls/reduce on DVE, delta/m/finals on GPSIMD,
Exp+Tanh on ACT (one table set - no ACT_TABLE_LOAD churn), input DMA on
sync, output DMA on scalar (second HWDGE ring).
"""

from contextlib import ExitStack
from math import log as _ln

import numpy as np

import concourse.bass as bass
import concourse.tile as tile
from concourse import bacc, mybir
from concourse.bass_utils import run_bass_kernel_spmd

N_RAYS = 131072
S = 128
N_CORES = 8
NC_RAYS = N_RAYS // N_CORES  # 16384 rays per core
BLOCK = 128                  # rays per partition-block
F32 = mybir.dt.float32
BF16 = mybir.dt.bfloat16
LN_HALF = _ln(0.5)


def build_nerf_bass(
    n_rays: int = NC_RAYS,
    t_blocks: int = 8,
    bufs: int = 2,
    mid_bufs: int = 0,
    gpsimd_dm: bool = True,
    gpsimd_finals: bool = False,
    tanh4: bool = True,
    fold: int = 0,
    reduce_stt: bool = False,
    out_dma: str = "scalar",
    g_psum: bool = False,
    cs_psum: bool = False,
    wg_psum: bool = False,
    scan_c1: bool = False,
) -> bass.Bass:
    T = t_blocks
    SUPER = BLOCK * T
    assert n_rays % SUPER == 0
    n_super = n_rays // SUPER
    N = S * T  # free extent per partition

    nc = bacc.Bacc("TRN2", target_bir_lowering=False, debug=False)
    # rgbo arrives channel-major [rays, 4, S]: kernel() transposes on the
    # host so opacity and each rgb channel are dense per ray on-chip.
    rgbo_h = nc.declare_dram_parameter("rgbo", [n_rays, 4, S], F32, isOutput=False)
    depth_h = nc.declare_dram_parameter("depth", [n_rays, S], F32, isOutput=False)
    out_h = nc.declare_dram_parameter("out", [n_rays, 3], F32, isOutput=True)

    rgbo_ap = rgbo_h.ap()
    depth_ap = depth_h.ap()
    out_ap = out_h.ap()

    with ExitStack() as ctx:
        tc = ctx.enter_context(tile.TileContext(nc))
        p_in = ctx.enter_context(tc.tile_pool(name="inp", bufs=bufs))
        p_mid = ctx.enter_context(tc.tile_pool(name="mid", bufs=mid_bufs or bufs))
        p_out = ctx.enter_context(tc.tile_pool(name="outp", bufs=bufs))
        p_ps = (
            ctx.enter_context(tc.tile_pool(name="ps", bufs=1, space="PSUM"))
            if (g_psum or cs_psum or wg_psum)
            else None
        )

        eng_dm = nc.gpsimd if gpsimd_dm else nc.vector
        eng_fin = nc.gpsimd if gpsimd_finals else nc.vector
        eng_odma = {"scalar": nc.scalar, "sync": nc.sync, "gpsimd": nc.gpsimd}[
            out_dma
        ]

        def emit_superblock(r0):
            rgbo_t = p_in.tile([BLOCK, 4 * N], F32, tag="rgbo")
            depth_t = p_in.tile([BLOCK, N], F32, tag="depth")
            nc.sync.dma_start(
                out=rgbo_t,
                in_=rgbo_ap[r0 : r0 + SUPER].rearrange(
                    "(p t) c s -> p (t c s)", p=BLOCK
                ),
            )
            nc.sync.dma_start(
                out=depth_t,
                in_=depth_ap[r0 : r0 + SUPER].rearrange(
                    "(p t) s -> p (t s)", p=BLOCK
                ),
            )

            depth3 = depth_t.rearrange("p (t s) -> p t s", t=T)
            rgbo3 = rgbo_t.rearrange("p (t x) -> p t x", t=T)  # x = (c s)
            rgb_in = rgbo3[:, :, 0 : 3 * S]  # dense 384-runs per ray
            o3 = rgbo3[:, :, 3 * S : 4 * S]  # opacity, dense per ray

            # delta / m (segmented; last sample of each ray zeroed so one
            # long scan never crosses a ray boundary with the sentinel)
            delta_t = p_mid.tile([BLOCK, N], F32, tag="delta")
            delta3 = delta_t.rearrange("p (t s) -> p t s", t=T)
            eng_dm.tensor_sub(
                delta3[:, :, 0 : S - 1], depth3[:, :, 1:S], depth3[:, :, 0 : S - 1]
            )
            m_t = p_mid.tile([BLOCK, N], F32, tag="m")
            m3 = m_t.rearrange("p (t s) -> p t s", t=T)
            eng_dm.tensor_mul(
                m3[:, :, 0 : S - 1],
                delta3[:, :, 0 : S - 1],
                o3[:, :, 0 : S - 1],
            )
            eng_dm.memset(m3[:, :, S - 1], 0.0)

            # one inclusive scan over the whole T*S extent
            cs_pool = p_ps if cs_psum else p_mid
            cs_t = cs_pool.tile([BLOCK, N], F32, tag="cs")
            scan_d1 = (
                nc.const_aps.tensor(0.0, (BLOCK, N)) if scan_c1 else m_t[:]
            )
            nc.vector.tensor_tensor_scan(
                cs_t[:],
                m_t[:],
                scan_d1,
                0.0,
                mybir.AluOpType.add,
                mybir.AluOpType.bypass,
            )

            # te[j] = exp(-cs[j-1]), te[0] = 1
            te_t = p_mid.tile([BLOCK, N + 4], F32, tag="te")
            eng_fin.memset(te_t[:, 0:1], 1.0)
            nc.scalar.activation(
                te_t[:, 1 : N + 1],
                cs_t[:],
                mybir.ActivationFunctionType.Exp,
                scale=-1.0,
            )

            # per-ray un-normalizer b[t] = exp(+cs[t*S-1]); the sigmoid
            # 0.5-affine is folded into the final tensor_scalar instead.
            cs3 = cs_t.rearrange("p (t s) -> p t s", t=T)
            b_t = p_mid.tile([BLOCK, T], F32, tag="b")
            eng_fin.memset(b_t[:, 0:1], 1.0)
            nc.scalar.activation(
                b_t[:, 1:T],
                cs3[:, 0 : T - 1, S - 1],
                mybir.ActivationFunctionType.Exp,
            )

            # g = tanh(rgb/2) in one ACT op: channel-major input means dense
            # 384-elem runs per ray on both sides (1 elem/cycle on ACT).
            # Layout (t c s): per-channel views are dense 128-runs -> the
            # bf16 wg muls get the 2x DVE mode.
            g_t = (p_ps if g_psum else p_mid).tile([BLOCK, 3 * N], BF16, tag="g")
            nc.scalar.activation(
                g_t.rearrange("p (t x) -> p t x", t=T),
                rgb_in,
                mybir.ActivationFunctionType.Tanh,
                scale=0.5,
            )
            # g_t layout (t c s): view as [p, c, t, s] (strides c->S, t->3S)
            gv = g_t.rearrange("p (t c s) -> p c t s", t=T, c=3)
            g_views = [gv[:, c] for c in range(3)]

            # w~[j] = te[j]-te[j+1] (zero at ray boundaries since m=0 there),
            # then overwrite each ray's last sample with te itself (alpha=1
            # under the FAR sentinel).
            w_t = p_mid.tile([BLOCK, N], BF16, tag="w")
            nc.vector.tensor_sub(w_t[:], te_t[:, 0:N], te_t[:, 1 : N + 1])
            te3 = te_t[:, 0:N].rearrange("p (t s) -> p t s", t=T)
            w3 = w_t.rearrange("p (t s) -> p t s", t=T)
            nc.vector.tensor_copy(w3[:, :, S - 1], te3[:, :, S - 1])

            # wg[c] = w~*g[c]  (all dense bf16 -> 2x DVE mode)
            wg_t = (p_ps if wg_psum else p_mid).tile([BLOCK, 3 * N], BF16, tag="wg")
            wg3 = wg_t.rearrange("p (c t s) -> p c t s", c=3, t=T)
            wts = w_t.rearrange("p (t s) -> p t s", t=T)
            for c in range(3):
                nc.vector.tensor_mul(wg3[:, c], wts, g_views[c])

            # segmented reduce over s -> S_c[t], layout [p, (c t)], with
            # `fold` levels of bf16 pairwise adds (2x mode) shrinking the
            # 1x-only tensor_reduce input first.
            s_t = p_mid.tile([BLOCK, 3 * T], F32, tag="s")
            red_in, seg = wg_t, S
            for lvl in range(fold):
                half = seg // 2
                f_t = p_mid.tile([BLOCK, 3 * T * half], BF16, tag=f"wgf{lvl}")
                nc.vector.tensor_add(
                    f_t.rearrange("p (n s) -> p n s", s=half),
                    red_in.rearrange("p (n s) -> p n s", s=seg)[:, :, 0:half],
                    red_in.rearrange("p (n s) -> p n s", s=seg)[:, :, half:seg],
                )
                red_in, seg = f_t, half
            nc.vector.tensor_reduce(
                s_t[:],
                red_in.rearrange("p (n s) -> p n s", s=seg),
                mybir.AxisListType.X,
                mybir.AluOpType.add,
            )

            # out[t,c] = (S_c[t]*b[t])*0.5 + 0.5
            out_t = p_out.tile([BLOCK, 3 * T], F32, tag="out")
            out3 = out_t.rearrange("p (t c) -> p t c", c=3)
            s3 = s_t.rearrange("p (c t) -> p c t", c=3)
            for c in range(3):
                eng_fin.tensor_mul(out3[:, :, c], s3[:, c], b_t[:])
            out2_t = p_out.tile([BLOCK, 3 * T], F32, tag="out2")
            eng_fin.tensor_scalar(
                out2_t[:],
                out_t[:],
                0.5,
                0.5,
                mybir.AluOpType.mult,
                mybir.AluOpType.add,
            )

            eng_odma.dma_start(
                out=out_ap[r0 : r0 + SUPER].rearrange("(p t) c -> p (t c)", p=BLOCK),
                in_=out2_t[:],
            )

        for sb in range(n_super):
            emit_superblock(sb * SUPER)
    nc.compile()
    return nc


_NC_CACHE: dict = {}


def _get_nc(**kwargs):
    key = tuple(sorted(kwargs.items()))
    if key not in _NC_CACHE:
        _NC_CACHE[key] = build_nerf_bass(**kwargs)
    return _NC_CACHE[key]


def kernel(rgbo: np.ndarray, depth: np.ndarray, build_kwargs=None, **run_kwargs) -> np.ndarray:
    rgbo = np.ascontiguousarray(rgbo, dtype=np.float32)
    depth = np.ascontiguousarray(depth, dtype=np.float32)
    assert rgbo.shape == (N_RAYS, S, 4) and depth.shape == (N_RAYS, S)

    nc = _get_nc(**(build_kwargs or {}))
    # channel-major layout so each rgb channel / opacity is dense per ray
    rgbo_cm = np.ascontiguousarray(rgbo.transpose(0, 2, 1))
    in_maps = []
    for i in range(N_CORES):
        sl = slice(i * NC_RAYS, (i + 1) * NC_RAYS)
        in_maps.append({"rgbo": rgbo_cm[sl], "depth": depth[sl]})
    res = run_bass_kernel_spmd(nc, in_maps, core_ids=list(range(N_CORES)), **run_kwargs)
    out = np.concatenate([r["out"] for r in res.results], axis=0)
    if run_kwargs:
        kernel.last_results = res  # stash for profiling harnesses
    return out


# revision 25
# speedup vs baseline: 1.4914x; 1.0046x over previous
"""NeRF volumetric alpha-compositing kernel for Trainium2 (Bass/Tile).

Full inputs:  rgbo [131072, 128, 4] f32, depth [131072, 128] f32.
Full output:  [131072, 3] f32.

Sharding: data-parallel over rays, 8 cores x 16384 rays.

Per-core algorithm, ray-per-partition layout (BLOCK=128 rays on partitions,
T rays per partition per superblock, S=128 samples each on the free dim):

  delta[s] = depth[s+1]-depth[s]; m[s] = opacity[s]*delta[s]; m[S-1] = 0
  cs       = inclusive_cumsum(m) over the whole T*S free extent (one scan;
             zeroing the last-sample sentinel keeps rays within a partition
             row from poisoning each other)
  te[j]    = exp(-cs[j-1]), te[0] = 1        (ACT Exp, scale=-1)
  b'[t]    = 0.5*exp(+cs[t*S-1])             (per-ray un-normalizer; exp
             bias=ln(1/2) folds the sigmoid->tanh affine)
  w~[j]    = te[j]-te[j+1]; w~[t,S-1] = te[t*S+S-1]   (bf16)
  g[c]     = tanh(0.5*rgb_c)                 (bf16; sigmoid(x) =
             0.5+0.5*tanh(x/2) keeps ACT on one table set with Exp)
  S_c[t]   = sum_s w~[t,s]*g[c][t,s]         (bf16 muls + segmented reduce)
  out[t,c] = S_c[t]*b'[t] + 0.5

The per-ray transmittance factor exp(+cs[ray start-1]) cancels the cross-ray
accumulation of the single long scan; sum_s w~ telescopes to 1/b exactly, so
the sigmoid "+0.5" term reduces to the constant 0.5.

Engine split: scan/w~/muls/reduce on DVE, delta/m/finals on GPSIMD,
Exp+Tanh on ACT (one table set - no ACT_TABLE_LOAD churn), input DMA on
sync, output DMA on scalar (second HWDGE ring).
"""

from contextlib import ExitStack
from math import log as _ln

import numpy as np

import concourse.bass as bass
import concourse.tile as tile
from concourse import bacc, mybir
from concourse.bass_utils import run_bass_kernel_spmd

N_RAYS = 131072
S = 128
N_CORES = 8
NC_RAYS = N_RAYS // N_CORES  # 16384 rays per core
BLOCK = 128                  # rays per partition-block
F32 = mybir.dt.float32
BF16 = mybir.dt.bfloat16
LN_HALF = _ln(0.5)


def build_nerf_bass(
    n_rays: int = NC_RAYS,
    t_blocks: int = 8,
    bufs: int = 2,
    mid_bufs: int = 0,
    gpsimd_dm: bool = True,
    gpsimd_finals: bool = False,
    tanh4: bool = True,
    fold: int = 0,
    reduce_stt: bool = False,
    out_dma: str = "scalar",
    g_psum: bool = False,
    cs_psum: bool = False,
    wg_psum: bool = False,
    scan_c1: bool = False,
) -> bass.Bass:
    T = t_blocks
    SUPER = BLOCK * T
    assert n_rays % SUPER == 0
    n_super = n_rays // SUPER
    N = S * T  # free extent per partition

    nc = bacc.Bacc("TRN2", target_bir_lowering=False, debug=False)
    # rgbo arrives channel-major [rays, 4, S]: kernel() transposes on the
    # host so opacity and each rgb channel are dense per ray on-chip.
    rgbo_h = nc.declare_dram_parameter("rgbo", [n_rays, 4, S], F32, isOutput=False)
    depth_h = nc.declare_dram_parameter("depth", [n_rays, S], F32, isOutput=False)
    out_h = nc.declare_dram_parameter("out", [n_rays, 3], F32, isOutput=True)

    rgbo_ap = rgbo_h.ap()
    depth_ap = depth_h.ap()
    out_ap = out_h.ap()

    with ExitStack() as ctx:
        tc = ctx.enter_context(tile.TileContext(nc))
        p_in = ctx.enter_context(tc.tile_pool(name="inp", bufs=bufs))
        p_mid = ctx.enter_context(tc.tile_pool(name="mid", bufs=mid_bufs or bufs))
        p_out = ctx.enter_context(tc.tile_pool(name="outp", bufs=bufs))
        p_ps = (
            ctx.enter_context(tc.tile_pool(name="ps", bufs=1, space="PSUM"))
            if (g_psum or cs_psum or wg_psum)
            else None
        )

        eng_dm = nc.gpsimd if gpsimd_dm else nc.vector
        eng_fin = nc.gpsimd if gpsimd_finals else nc.vector
        eng_odma = {"scalar": nc.scalar, "sync": nc.sync, "gpsimd": nc.gpsimd}[
            out_dma
        ]

        def emit_head(r0):
            """DMA in, tanh, delta/m, scan. Only depends on this sb's DMA."""
            rgbo_t = p_in.tile([BLOCK, 4 * N], F32, tag="rgbo")
            depth_t = p_in.tile([BLOCK, N], F32, tag="depth")
            nc.sync.dma_start(
                out=rgbo_t,
                in_=rgbo_ap[r0 : r0 + SUPER].rearrange(
                    "(p t) c s -> p (t c s)", p=BLOCK
                ),
            )
            nc.sync.dma_start(
                out=depth_t,
                in_=depth_ap[r0 : r0 + SUPER].rearrange(
                    "(p t) s -> p (t s)", p=BLOCK
                ),
            )

            depth3 = depth_t.rearrange("p (t s) -> p t s", t=T)
            rgbo3 = rgbo_t.rearrange("p (t x) -> p t x", t=T)  # x = (c s)
            rgb_in = rgbo3[:, :, 0 : 3 * S]  # dense 384-runs per ray
            o3 = rgbo3[:, :, 3 * S : 4 * S]  # opacity, dense per ray

            # g = tanh(rgb/2) in one ACT op: channel-major input means dense
            # 384-elem runs per ray on both sides (1 elem/cycle on ACT).
            # Layout (t c s): per-channel views are dense 128-runs -> the
            # bf16 wg muls get the 2x DVE mode. Emitted FIRST on ACT so it
            # overlaps this sb's scan instead of stalling behind exp.
            g_t = (p_ps if g_psum else p_mid).tile([BLOCK, 3 * N], BF16, tag="g")
            nc.scalar.activation(
                g_t.rearrange("p (t x) -> p t x", t=T),
                rgb_in,
                mybir.ActivationFunctionType.Tanh,
                scale=0.5,
            )

            # delta / m (segmented; last sample of each ray zeroed so one
            # long scan never crosses a ray boundary with the sentinel)
            delta_t = p_mid.tile([BLOCK, N], F32, tag="delta")
            delta3 = delta_t.rearrange("p (t s) -> p t s", t=T)
            eng_dm.tensor_sub(
                delta3[:, :, 0 : S - 1], depth3[:, :, 1:S], depth3[:, :, 0 : S - 1]
            )
            m_t = p_mid.tile([BLOCK, N], F32, tag="m")
            m3 = m_t.rearrange("p (t s) -> p t s", t=T)
            eng_dm.tensor_mul(
                m3[:, :, 0 : S - 1],
                delta3[:, :, 0 : S - 1],
                o3[:, :, 0 : S - 1],
            )
            eng_dm.memset(m3[:, :, S - 1], 0.0)

            # one inclusive scan over the whole T*S extent
            cs_t = (p_ps if cs_psum else p_mid).tile([BLOCK, N], F32, tag="cs")
            scan_d1 = (
                nc.const_aps.tensor(0.0, (BLOCK, N)) if scan_c1 else m_t[:]
            )
            nc.vector.tensor_tensor_scan(
                cs_t[:],
                m_t[:],
                scan_d1,
                0.0,
                mybir.AluOpType.add,
                mybir.AluOpType.bypass,
            )
            return r0, g_t, cs_t

        def emit_tail(state):
            r0, g_t, cs_t = state
            # te[j] = exp(-cs[j-1]), te[0] = 1
            te_t = p_mid.tile([BLOCK, N + 4], F32, tag="te")
            nc.vector.memset(te_t[:, 0:1], 1.0)
            nc.scalar.activation(
                te_t[:, 1 : N + 1],
                cs_t[:],
                mybir.ActivationFunctionType.Exp,
                scale=-1.0,
            )

            # per-ray un-normalizer b[t] = exp(+cs[t*S-1]); the sigmoid
            # 0.5-affine is folded into the final tensor_scalar instead.
            cs3 = cs_t.rearrange("p (t s) -> p t s", t=T)
            b_t = p_mid.tile([BLOCK, T], F32, tag="b")
            nc.vector.memset(b_t[:, 0:1], 1.0)
            nc.scalar.activation(
                b_t[:, 1:T],
                cs3[:, 0 : T - 1, S - 1],
                mybir.ActivationFunctionType.Exp,
            )

            # w~[j] = te[j]-te[j+1] (zero at ray boundaries since m=0 there),
            # then overwrite each ray's last sample with te itself (alpha=1
            # under the FAR sentinel).
            w_t = p_mid.tile([BLOCK, N], BF16, tag="w")
            nc.vector.tensor_sub(w_t[:], te_t[:, 0:N], te_t[:, 1 : N + 1])
            te3 = te_t[:, 0:N].rearrange("p (t s) -> p t s", t=T)
            w3 = w_t.rearrange("p (t s) -> p t s", t=T)
            nc.vector.tensor_copy(w3[:, :, S - 1], te3[:, :, S - 1])

            # wg[c] = w~*g[c]  (all dense bf16 -> 2x DVE mode)
            gv = g_t.rearrange("p (t c s) -> p c t s", t=T, c=3)
            wg_t = (p_ps if wg_psum else p_mid).tile([BLOCK, 3 * N], BF16, tag="wg")
            wg3 = wg_t.rearrange("p (c t s) -> p c t s", c=3, t=T)
            wts = w_t.rearrange("p (t s) -> p t s", t=T)
            for c in range(3):
                nc.vector.tensor_mul(wg3[:, c], wts, gv[:, c])

            # segmented reduce over s -> S_c[t], layout [p, (c t)], with
            # `fold` levels of bf16 pairwise adds (2x mode) shrinking the
            # 1x-only tensor_reduce input first.
            s_t = p_mid.tile([BLOCK, 3 * T], F32, tag="s")
            red_in, seg = wg_t, S
            for lvl in range(fold):
                half = seg // 2
                f_t = p_mid.tile([BLOCK, 3 * T * half], BF16, tag=f"wgf{lvl}")
                nc.vector.tensor_add(
                    f_t.rearrange("p (n s) -> p n s", s=half),
                    red_in.rearrange("p (n s) -> p n s", s=seg)[:, :, 0:half],
                    red_in.rearrange("p (n s) -> p n s", s=seg)[:, :, half:seg],
                )
                red_in, seg = f_t, half
            nc.vector.tensor_reduce(
                s_t[:],
                red_in.rearrange("p (n s) -> p n s", s=seg),
                mybir.AxisListType.X,
                mybir.AluOpType.add,
            )

            # out[t,c] = (S_c[t]*b[t])*0.5 + 0.5
            out_t = p_out.tile([BLOCK, 3 * T], F32, tag="out")
            out3 = out_t.rearrange("p (t c) -> p t c", c=3)
            s3 = s_t.rearrange("p (c t) -> p c t", c=3)
            for c in range(3):
                eng_fin.tensor_mul(out3[:, :, c], s3[:, c], b_t[:])
            out2_t = p_out.tile([BLOCK, 3 * T], F32, tag="out2")
            eng_fin.tensor_scalar(
                out2_t[:],
                out_t[:],
                0.5,
                0.5,
                mybir.AluOpType.mult,
                mybir.AluOpType.add,
            )

            eng_odma.dma_start(
                out=out_ap[r0 : r0 + SUPER].rearrange("(p t) c -> p (t c)", p=BLOCK),
                in_=out2_t[:],
            )

        # software pipeline: HEAD(n+1) is emitted before TAIL(n) so every
        # engine queue holds independent work between dependent pairs
        # (in-order queues otherwise stall on cross-engine round trips).
        pending = emit_head(0)
        for sb in range(1, n_super):
            nxt = emit_head(sb * SUPER)
            emit_tail(pending)
            pending = nxt
        emit_tail(pending)
    nc.compile()
    return nc


_NC_CACHE: dict = {}


def _get_nc(**kwargs):
    key = tuple(sorted(kwargs.items()))
    if key not in _NC_CACHE:
        _NC_CACHE[key] = build_nerf_bass(**kwargs)
    return _NC_CACHE[key]


def kernel(rgbo: np.ndarray, depth: np.ndarray, build_kwargs=None, **run_kwargs) -> np.ndarray:
    rgbo = np.ascontiguousarray(rgbo, dtype=np.float32)
    depth = np.ascontiguousarray(depth, dtype=np.float32)
    assert rgbo.shape == (N_RAYS, S, 4) and depth.shape == (N_RAYS, S)

    nc = _get_nc(**(build_kwargs or {}))
    # channel-major layout so each rgb channel / opacity is dense per ray
    rgbo_cm = np.ascontiguousarray(rgbo.transpose(0, 2, 1))
    in_maps = []
    for i in range(N_CORES):
        sl = slice(i * NC_RAYS, (i + 1) * NC_RAYS)
        in_maps.append({"rgbo": rgbo_cm[sl], "depth": depth[sl]})
    res = run_bass_kernel_spmd(nc, in_maps, core_ids=list(range(N_CORES)), **run_kwargs)
    out = np.concatenate([r["out"] for r in res.results], axis=0)
    if run_kwargs:
        kernel.last_results = res  # stash for profiling harnesses
    return out


# revision 28
# speedup vs baseline: 1.4966x; 1.0035x over previous
"""NeRF volumetric alpha-compositing kernel for Trainium2 (Bass/Tile).

Full inputs:  rgbo [131072, 128, 4] f32, depth [131072, 128] f32.
Full output:  [131072, 3] f32.

Sharding: data-parallel over rays, 8 cores x 16384 rays.

Per-core algorithm, ray-per-partition layout (BLOCK=128 rays on partitions,
T rays per partition per superblock, S=128 samples each on the free dim):

  delta[s] = depth[s+1]-depth[s]; m[s] = opacity[s]*delta[s]; m[S-1] = 0
  cs       = inclusive_cumsum(m) over the whole T*S free extent (one scan;
             zeroing the last-sample sentinel keeps rays within a partition
             row from poisoning each other)
  te[j]    = exp(-cs[j-1]), te[0] = 1        (ACT Exp, scale=-1)
  b'[t]    = 0.5*exp(+cs[t*S-1])             (per-ray un-normalizer; exp
             bias=ln(1/2) folds the sigmoid->tanh affine)
  w~[j]    = te[j]-te[j+1]; w~[t,S-1] = te[t*S+S-1]   (bf16)
  g[c]     = tanh(0.5*rgb_c)                 (bf16; sigmoid(x) =
             0.5+0.5*tanh(x/2) keeps ACT on one table set with Exp)
  S_c[t]   = sum_s w~[t,s]*g[c][t,s]         (bf16 muls + segmented reduce)
  out[t,c] = S_c[t]*b'[t] + 0.5

The per-ray transmittance factor exp(+cs[ray start-1]) cancels the cross-ray
accumulation of the single long scan; sum_s w~ telescopes to 1/b exactly, so
the sigmoid "+0.5" term reduces to the constant 0.5.

Engine split: scan/w~/muls/reduce on DVE, delta/m/finals on GPSIMD,
Exp+Tanh on ACT (one table set - no ACT_TABLE_LOAD churn), input DMA on
sync, output DMA on scalar (second HWDGE ring).
"""

from contextlib import ExitStack
from math import log as _ln

import numpy as np

import concourse.bass as bass
import concourse.tile as tile
from concourse import bacc, mybir
from concourse.bass_utils import run_bass_kernel_spmd

N_RAYS = 131072
S = 128
N_CORES = 8
NC_RAYS = N_RAYS // N_CORES  # 16384 rays per core
BLOCK = 128                  # rays per partition-block
F32 = mybir.dt.float32
BF16 = mybir.dt.bfloat16
LN_HALF = _ln(0.5)


def build_nerf_bass(
    n_rays: int = NC_RAYS,
    t_blocks: int = 8,
    bufs: int = 2,
    mid_bufs: int = 0,
    gpsimd_dm: bool = True,
    gpsimd_finals: bool = False,
    tanh4: bool = True,
    fold: int = 0,
    reduce_stt: bool = False,
    out_dma: str = "scalar",
    g_psum: bool = False,
    cs_psum: bool = False,
    wg_psum: bool = False,
    scan_c1: bool = False,
) -> bass.Bass:
    T = t_blocks
    SUPER = BLOCK * T
    assert n_rays % SUPER == 0
    n_super = n_rays // SUPER
    N = S * T  # free extent per partition

    nc = bacc.Bacc("TRN2", target_bir_lowering=False, debug=False)
    # rgbo arrives channel-major [rays, 4, S]: kernel() transposes on the
    # host so opacity and each rgb channel are dense per ray on-chip.
    rgbo_h = nc.declare_dram_parameter("rgbo", [n_rays, 4, S], F32, isOutput=False)
    depth_h = nc.declare_dram_parameter("depth", [n_rays, S], F32, isOutput=False)
    out_h = nc.declare_dram_parameter("out", [n_rays, 3], F32, isOutput=True)

    rgbo_ap = rgbo_h.ap()
    depth_ap = depth_h.ap()
    out_ap = out_h.ap()

    with ExitStack() as ctx:
        tc = ctx.enter_context(tile.TileContext(nc))
        p_in = ctx.enter_context(tc.tile_pool(name="inp", bufs=bufs))
        p_mid = ctx.enter_context(tc.tile_pool(name="mid", bufs=mid_bufs or bufs))
        p_out = ctx.enter_context(tc.tile_pool(name="outp", bufs=bufs))
        p_ps = (
            ctx.enter_context(tc.tile_pool(name="ps", bufs=bufs, space="PSUM"))
            if (g_psum or cs_psum or wg_psum)
            else None
        )

        eng_dm = nc.gpsimd if gpsimd_dm else nc.vector
        eng_fin = nc.gpsimd if gpsimd_finals else nc.vector
        eng_odma = {"scalar": nc.scalar, "sync": nc.sync, "gpsimd": nc.gpsimd}[
            out_dma
        ]

        def emit_head(r0):
            """DMA in, tanh, delta/m, scan. Only depends on this sb's DMA."""
            rgbo_t = p_in.tile([BLOCK, 4 * N], F32, tag="rgbo")
            depth_t = p_in.tile([BLOCK, N], F32, tag="depth")
            nc.sync.dma_start(
                out=rgbo_t,
                in_=rgbo_ap[r0 : r0 + SUPER].rearrange(
                    "(p t) c s -> p (t c s)", p=BLOCK
                ),
            )
            nc.sync.dma_start(
                out=depth_t,
                in_=depth_ap[r0 : r0 + SUPER].rearrange(
                    "(p t) s -> p (t s)", p=BLOCK
                ),
            )

            depth3 = depth_t.rearrange("p (t s) -> p t s", t=T)
            rgbo3 = rgbo_t.rearrange("p (t x) -> p t x", t=T)  # x = (c s)
            rgb_in = rgbo3[:, :, 0 : 3 * S]  # dense 384-runs per ray
            o3 = rgbo3[:, :, 3 * S : 4 * S]  # opacity, dense per ray

            # g = tanh(rgb/2) in one ACT op: channel-major input means dense
            # 384-elem runs per ray on both sides (1 elem/cycle on ACT).
            # Layout (t c s): per-channel views are dense 128-runs -> the
            # bf16 wg muls get the 2x DVE mode. Emitted FIRST on ACT so it
            # overlaps this sb's scan instead of stalling behind exp.
            g_t = (p_ps if g_psum else p_mid).tile([BLOCK, 3 * N], BF16, tag="g")
            nc.scalar.activation(
                g_t.rearrange("p (t x) -> p t x", t=T),
                rgb_in,
                mybir.ActivationFunctionType.Tanh,
                scale=0.5,
            )

            # delta / m (segmented; last sample of each ray zeroed so one
            # long scan never crosses a ray boundary with the sentinel)
            delta_t = p_mid.tile([BLOCK, N], F32, tag="delta")
            delta3 = delta_t.rearrange("p (t s) -> p t s", t=T)
            eng_dm.tensor_sub(
                delta3[:, :, 0 : S - 1], depth3[:, :, 1:S], depth3[:, :, 0 : S - 1]
            )
            m_t = p_mid.tile([BLOCK, N], F32, tag="m")
            m3 = m_t.rearrange("p (t s) -> p t s", t=T)
            eng_dm.tensor_mul(
                m3[:, :, 0 : S - 1],
                delta3[:, :, 0 : S - 1],
                o3[:, :, 0 : S - 1],
            )
            # soft sentinel: te drops by e^-8 at each ray boundary, so the
            # plain w~ difference already yields the last sample's weight
            # (te[last], to 3e-4 relative) and no fix-up op is needed. The
            # per-ray un-normalizer b = exp(+cs_boundary) stays finite:
            # cs <= 7*(4+8) = 84 < ln(f32max), and bf16 wg values stay
            # above bf16's min normal (e^-84 = 3e-37 > 1.2e-38).
            eng_dm.memset(m3[:, :, S - 1], 8.0)

            # one inclusive scan over the whole T*S extent
            cs_t = (p_ps if cs_psum else p_mid).tile([BLOCK, N], F32, tag="cs")
            scan_d1 = (
                nc.const_aps.tensor(0.0, (BLOCK, N)) if scan_c1 else m_t[:]
            )
            nc.vector.tensor_tensor_scan(
                cs_t[:],
                m_t[:],
                scan_d1,
                0.0,
                mybir.AluOpType.add,
                mybir.AluOpType.bypass,
            )
            return r0, g_t, cs_t

        def emit_tail(state):
            r0, g_t, cs_t = state
            # te[j] = exp(-cs[j-1]), te[0] = 1
            te_t = p_mid.tile([BLOCK, N + 4], F32, tag="te")
            nc.vector.memset(te_t[:, 0:1], 1.0)
            nc.scalar.activation(
                te_t[:, 1 : N + 1],
                cs_t[:],
                mybir.ActivationFunctionType.Exp,
                scale=-1.0,
            )

            # per-ray un-normalizer b[t] = exp(+cs[t*S-1]); the sigmoid
            # 0.5-affine is folded into the final tensor_scalar instead.
            cs3 = cs_t.rearrange("p (t s) -> p t s", t=T)
            b_t = p_mid.tile([BLOCK, T], F32, tag="b")
            nc.vector.memset(b_t[:, 0:1], 1.0)
            nc.scalar.activation(
                b_t[:, 1:T],
                cs3[:, 0 : T - 1, S - 1],
                mybir.ActivationFunctionType.Exp,
            )

            # w~[j] = te[j]-te[j+1]; the soft sentinel makes each ray's last
            # entry come out as te[last]*(1-e^-8) with no fix-up op.
            w_t = p_mid.tile([BLOCK, N], BF16, tag="w")
            nc.vector.tensor_sub(w_t[:], te_t[:, 0:N], te_t[:, 1 : N + 1])

            # wg[c] = w~*g[c]  (all dense bf16 -> 2x DVE mode)
            gv = g_t.rearrange("p (t c s) -> p c t s", t=T, c=3)
            wg_t = (p_ps if wg_psum else p_mid).tile([BLOCK, 3 * N], BF16, tag="wg")
            wg3 = wg_t.rearrange("p (c t s) -> p c t s", c=3, t=T)
            wts = w_t.rearrange("p (t s) -> p t s", t=T)
            for c in range(3):
                nc.vector.tensor_mul(wg3[:, c], wts, gv[:, c])

            # segmented reduce over s -> S_c[t], layout [p, (c t)], with
            # `fold` levels of bf16 pairwise adds (2x mode) shrinking the
            # 1x-only tensor_reduce input first.
            s_t = p_mid.tile([BLOCK, 3 * T], F32, tag="s")
            red_in, seg = wg_t, S
            for lvl in range(fold):
                half = seg // 2
                f_t = p_mid.tile([BLOCK, 3 * T * half], BF16, tag=f"wgf{lvl}")
                nc.vector.tensor_add(
                    f_t.rearrange("p (n s) -> p n s", s=half),
                    red_in.rearrange("p (n s) -> p n s", s=seg)[:, :, 0:half],
                    red_in.rearrange("p (n s) -> p n s", s=seg)[:, :, half:seg],
                )
                red_in, seg = f_t, half
            nc.vector.tensor_reduce(
                s_t[:],
                red_in.rearrange("p (n s) -> p n s", s=seg),
                mybir.AxisListType.X,
                mybir.AluOpType.add,
            )

            # out[t,c] = (S_c[t]*b[t])*0.5 + 0.5
            out_t = p_out.tile([BLOCK, 3 * T], F32, tag="out")
            out3 = out_t.rearrange("p (t c) -> p t c", c=3)
            s3 = s_t.rearrange("p (c t) -> p c t", c=3)
            for c in range(3):
                eng_fin.tensor_mul(out3[:, :, c], s3[:, c], b_t[:])
            out2_t = p_out.tile([BLOCK, 3 * T], F32, tag="out2")
            eng_fin.tensor_scalar(
                out2_t[:],
                out_t[:],
                0.5,
                0.5,
                mybir.AluOpType.mult,
                mybir.AluOpType.add,
            )

            eng_odma.dma_start(
                out=out_ap[r0 : r0 + SUPER].rearrange("(p t) c -> p (t c)", p=BLOCK),
                in_=out2_t[:],
            )

        # software pipeline: HEAD(n+1) is emitted before TAIL(n) so every
        # engine queue holds independent work between dependent pairs
        # (in-order queues otherwise stall on cross-engine round trips).
        pending = emit_head(0)
        for sb in range(1, n_super):
            nxt = emit_head(sb * SUPER)
            emit_tail(pending)
            pending = nxt
        emit_tail(pending)
    nc.compile()
    return nc


_NC_CACHE: dict = {}


def _get_nc(**kwargs):
    key = tuple(sorted(kwargs.items()))
    if key not in _NC_CACHE:
        _NC_CACHE[key] = build_nerf_bass(**kwargs)
    return _NC_CACHE[key]


def kernel(rgbo: np.ndarray, depth: np.ndarray, build_kwargs=None, **run_kwargs) -> np.ndarray:
    rgbo = np.ascontiguousarray(rgbo, dtype=np.float32)
    depth = np.ascontiguousarray(depth, dtype=np.float32)
    assert rgbo.shape == (N_RAYS, S, 4) and depth.shape == (N_RAYS, S)

    nc = _get_nc(**(build_kwargs or {}))
    # channel-major layout so each rgb channel / opacity is dense per ray
    rgbo_cm = np.ascontiguousarray(rgbo.transpose(0, 2, 1))
    in_maps = []
    for i in range(N_CORES):
        sl = slice(i * NC_RAYS, (i + 1) * NC_RAYS)
        in_maps.append({"rgbo": rgbo_cm[sl], "depth": depth[sl]})
    res = run_bass_kernel_spmd(nc, in_maps, core_ids=list(range(N_CORES)), **run_kwargs)
    out = np.concatenate([r["out"] for r in res.results], axis=0)
    if run_kwargs:
        kernel.last_results = res  # stash for profiling harnesses
    return out
